# revision 49
# baseline (speedup 1.0000x reference)
"""GNN message-passing node model on 8 TRN2 NeuronCores.

Reference computation:
    agg = segment_sum(edge_attr, edge_index[1], num_segments=N)   # scatter-add
    h   = relu(concat([x, agg], 1) @ W1 + b1)
    out = h @ W2 + b2

Sharding: destination nodes are split into 8 contiguous blocks of 6250;
edges are partitioned by destination (per the sharding hint), so the
scatter-add is fully local per core -- no halo exchange.

Per core the segment-sum runs as dense TensorE matmuls over 128-edge tiles:
    aggT[128 feat, 64 nodes] += E_tile[128 edge, 128 feat]^T @ S_tile
with 64-node destination windows.  Each tile holds TWO rank levels of the
window (rank r edges at partitions 0-63, rank r+1 at 64-127), so the
selector for in-cap tiles is the constant stacked identity [I64; I64] and
each tile streams only 64 columns (the per-tile cost is LDWEIGHTS-bound,
~35 ns/tile with the fp8 fast-weight-load path).

Nodes are RELABELED on the host (x/out permuted to match): sorted by
descending degree into degree-homogeneous windows (~95% of edge-tile
slots used, nearly zero overflow tiles), then the windows are snake-dealt
across the 13 MLP groups so every group carries an equal share of edge
bytes (otherwise TensorE starves on the edge DMA early).  The rare
overflow edges get one-hot selectors built in bulk per group on VectorE
via is_equal(iota bcast, colrel bcast).

Edge values AND the resident x^T ride in fp8 e3m4 (x2 pre-scale, folded
back via W1b resp. W1a * 0.5 on the host -- exact exponent shifts): this
halves the dominant HBM traffic vs bf16 and keeps the end-to-end
relative error ~1.42e-2 (validated against the reference in numpy AND on
hardware; e3m4 has 4 mantissa bits, ample for N(0,1) data; W1a itself
must stay bf16 -- its ~+-0.06 entries fall into e3m4's subnormal range).
Selector products are exact (0/1 x fp8), accumulation is f32 PSUM, and
the W1a(bf16) x xT(fp8) matmul runs as a mixed-dtype matmul.

The MLP is software-pipelined ONE GROUP BEHIND the scatter, entirely
on-chip in bf16: the aggT PSUM->SBUF copy runs on ScalarE (off the
TensorE critical path), h = relu(W1a^T x^T + W1b'^T aggT + b1) with the
relu+bias as a single VectorE scalar_tensor_tensor (add, max 0) split in
halves so the W2 stage starts after half the activation, and the b2 bias
rides the PSUM->SBUF output copy (DVE add against a broadcast tile).

DMA plan (a hardware-DGE enqueue occupies its engine ~0.6-1.2 us, so the
compute-carrying Scalar engine gets only a handful): Sync carries the
bulk edge stream (groups 0-8) + MLP weights; Scalar carries consts +
resident x^T early plus two late edge groups enqueued between computes;
gpsimd (software DGE) prefetches the two tail groups and then streams the
outputs.  All 13 edge-group tiles are SBUF-resident (no ring-reuse waits).
The output stays partition-major [128, blk, H] in DRAM (fully contiguous
stores); the host transposes back to [node, feat] and undoes the degree
sort.
"""

import os
import sys
import types

import numpy as np
import ml_dtypes

N_NODES = 50000
N_EDGES = 600000
H = 128
N_CORES = 8
NPC = N_NODES // N_CORES          # 6250 nodes per core
WIN = 64                          # destination-node window (matmul N dim)
RANKS = 2                         # rank levels stacked per tile (64*2=128)
NW = (NPC + WIN - 1) // WIN       # 98 windows per core
NPAD = NW * WIN                   # 6272 padded nodes per core
NBLK = NPAD // H                  # 49 output blocks of 128 nodes
GROUP = 8                         # windows per MLP group (512 nodes)
KMAX = 48                         # max rank-cap considered
FP8_SCALE = 2.0                   # edge pre-scale (power of 2, exact fold)


def _install_axon_trace_shim():
    """If the harness sets BASS_TRACE=1, run_bass_kernel_spmd imports
    antenv.axon_hooks; slim axon containers lack it.  Provide the same
    ctypes-based NTFF hook trn_agent_boot would register, so tracing works
    instead of crashing.  No-op when the real module exists."""
    try:
        import antenv.axon_hooks  # noqa: F401
        return
    except ImportError:
        pass
    mod = types.ModuleType("antenv.axon_hooks")
    mod._hook = None
    mod.set_axon_ntff_profile_hook = lambda h: setattr(mod, "_hook", h)
    mod.get_axon_ntff_profile_hook = lambda: mod._hook
    sys.modules["antenv.axon_hooks"] = mod
    so_path = "/opt/axon/libaxon_pjrt.so"
    if os.path.exists(so_path):
        try:
            from trn_agent_boot.trn_boot import _ntff_profile_via_ctypes
            mod._hook = _ntff_profile_via_ctypes(so_path)
        except Exception:
            mod._hook = None
    try:
        from concourse import bass_utils
        _orig_upload = bass_utils.upload_artifacts

        def _safe_upload(tmpdir):
            try:
                return _orig_upload(tmpdir)
            except Exception as e:  # no bucket access in sandbox
                return f"upload-skipped({e.__class__.__name__})"

        bass_utils.upload_artifacts = _safe_upload
    except Exception:
        pass


def _prep_host(x, edge_index, edge_attr, W1, b1, W2, b2):
    """Rank-pack edges per (core, 64-node window); build per-core inputs."""
    fp8 = ml_dtypes.float8_e3m4
    bf16 = ml_dtypes.bfloat16

    col0 = np.asarray(edge_index)[1].astype(np.int64)
    deg0 = np.bincount(col0, minlength=N_NODES)
    # Permute nodes within each core (host-only relabeling; x/out are
    # permuted to match): sort by descending degree into degree-homogeneous
    # 64-node windows (~95% of edge slots used, almost no overflow tiles),
    # then snake-deal the windows across the MLP groups so every group
    # carries a near-equal share of edges -- otherwise the first groups
    # hold most of the bytes and TensorE starves on the edge DMA early on.
    n_groups_p = (NW + GROUP - 1) // GROUP
    gslots = [GROUP] * n_groups_p
    gslots[-1] = NW - GROUP * (n_groups_p - 1)
    perm = np.empty(N_NODES, np.int64)     # perm[new_id] = old_id
    for c in range(N_CORES):
        seg = slice(c * NPC, (c + 1) * NPC)
        order_nodes = np.argsort(-deg0[seg], kind="stable") + c * NPC
        wins = [order_nodes[64 * j:64 * j + 64] for j in range(NW)]
        partial = wins.pop()               # 42-node window pinned last
        ww = np.array([deg0[wn].sum() for wn in wins])
        worder = np.argsort(ww, kind="stable")          # light -> heavy
        groups = [[] for _ in range(n_groups_p)]
        gi, step = 0, 1
        for j in worder:                    # snake-deal for equal sums
            while len(groups[gi]) >= gslots[gi] - (1 if gi == n_groups_p - 1 else 0):
                gi += step
                if gi in (-1, n_groups_p):
                    step = -step
                    gi += step
            groups[gi].append(j)
            gi += step
            if gi in (-1, n_groups_p):
                step = -step
                gi += step
        dealt = [wins[j] for grp in groups for j in grp] + [partial]
        perm[seg] = np.concatenate(dealt)
    inv = np.empty(N_NODES, np.int64)
    inv[perm] = np.arange(N_NODES)
    col = inv[col0]
    core = col // NPC
    local = col - core * NPC               # 0..6249
    w = local // WIN                       # 0..97
    rel = local - w * WIN                  # 0..63

    # per-node degree and per-edge rank within its node
    deg = deg0[perm]
    node_start = np.concatenate([[0], np.cumsum(deg)[:-1]])
    order = np.argsort(col, kind="stable")         # edges grouped by node
    scol = col[order]
    rank = np.arange(N_EDGES, dtype=np.int64) - node_start[scol]

    # choose rank cap per window: minimize ceil(k/RANKS) + ceil(over/128)
    degs = np.zeros((N_CORES, NW * WIN), np.int64)
    degs[:, :NPC] = deg.reshape(N_CORES, NPC)
    degs = degs.reshape(N_CORES, NW, WIN)
    ks = np.arange(KMAX + 1)
    over = np.clip(degs[..., None] - ks, 0, None).sum(axis=2)   # [C, NW, K+1]
    over_max = over.max(axis=0)                                  # [NW, K+1]
    cap_ov_k = -(-over_max // 128)
    cost = -(-ks // RANKS) + cap_ov_k                            # [NW, K+1]
    kstar = np.zeros(NW, np.int64)
    for wi in range(NW):
        c = cost[wi]
        best = int(np.min(c))
        kstar[wi] = int(np.max(np.nonzero(c == best)[0]))
    cap_id = kstar                                    # rank cap (edges/node)
    n_id = -(-cap_id // RANKS)                        # identity tiles/window
    cap_ov = cap_ov_k[np.arange(NW), kstar]           # overflow tiles/window
    caps = np.maximum(1, n_id + cap_ov)               # total tiles/window
    tstart = np.concatenate([[0], np.cumsum(caps)[:-1]])
    T = int(caps.sum())

    # slot assignment (in node-sorted edge order)
    e_core = core[order]
    e_w = w[order]
    e_rel = rel[order]
    is_id = rank < cap_id[e_w]
    tile_g = np.empty(N_EDGES, np.int64)
    part = np.empty(N_EDGES, np.int64)
    tile_g[is_id] = tstart[e_w[is_id]] + rank[is_id] // RANKS
    part[is_id] = (rank[is_id] % RANKS) * WIN + e_rel[is_id]
    # overflow edges: ordinal within (core, window) block (node-order keeps
    # blocks contiguous)
    ovm = ~is_id
    ovkey = e_core[ovm] * NW + e_w[ovm]
    ovcnt = np.bincount(ovkey, minlength=N_CORES * NW)
    ovstart = np.concatenate([[0], np.cumsum(ovcnt)[:-1]])
    ov_rank = np.arange(int(ovm.sum()), dtype=np.int64) - ovstart[ovkey]
    tile_g[ovm] = tstart[e_w[ovm]] + n_id[e_w[ovm]] + ov_rank // 128
    part[ovm] = ov_rank % 128

    ea = np.clip(np.asarray(edge_attr, np.float32) * FP8_SCALE, -15.5, 15.5)
    ea = ea.astype(fp8)
    x = np.asarray(x, np.float32)
    W1 = np.asarray(W1, np.float32)
    b1 = np.asarray(b1, np.float32)
    W2 = np.asarray(W2, np.float32)
    b2 = np.asarray(b2, np.float32)

    iota = np.tile(np.arange(WIN, dtype=np.float32), (128, 1)).astype(bf16)
    ident2 = np.zeros((128, WIN), np.float32)
    ident2[np.arange(128), np.arange(128) % WIN] = 1.0   # [I64; I64]
    ident2 = ident2.astype(fp8)
    w1a = np.ascontiguousarray(W1[:H] * (1.0 / FP8_SCALE)).astype(bf16)
    w1b = np.ascontiguousarray(W1[H:] * (1.0 / FP8_SCALE)).astype(bf16)
    w2c = W2.astype(bf16)
    b1c = b1.reshape(H, 1).astype(bf16)                     # ACT bias column
    # b2 broadcast across partitions: fused into the PSUM->SBUF copy on DVE
    b2bc = np.tile(b2.reshape(1, H), (128, GROUP * WIN // H)).astype(bf16)

    # overflow tiles: global tile index -> compact overflow column index
    ov_tiles = []
    for wi in range(NW):
        for t in range(int(n_id[wi]), int(caps[wi])):
            ov_tiles.append(int(tstart[wi]) + t)
    NOV = max(1, len(ov_tiles))
    ov_col = np.full(T, -1, np.int64)
    for j, tg in enumerate(ov_tiles):
        ov_col[tg] = j

    in_maps = []
    for c in range(N_CORES):
        m = (e_core == c)
        edges_c = np.zeros((128, T, H), dtype=fp8)
        edges_c[part[m], tile_g[m], :] = ea[order[m]]
        colrel_c = np.full((128, NOV), 200.0, np.float32).astype(bf16)
        mo = m & ovm
        colrel_c[part[mo], ov_col[tile_g[mo]]] = e_rel[mo]
        # bf16 consts in wide blobs: hardware-DGE DMA cost is packet-count
        # bound (~128 packets per transfer regardless of width), so wide
        # transfers beat narrow ones ~8x.  cbA feeds the selector builds
        # (scalar queue, early); cbW feeds the MLP (sync queue, right after
        # group-0 edges so it beats MLP(0) despite queue contention).
        cba_c = np.concatenate([iota, colrel_c, b2bc], axis=1)
        cbw_c = np.concatenate([w1a, w1b, w2c, b1c], axis=1)
        xT_c = np.zeros((H, NPAD), dtype=fp8)
        xT_c[:, :NPC] = np.clip(
            x[perm[c * NPC:(c + 1) * NPC]].T * FP8_SCALE, -15.5, 15.5).astype(fp8)
        in_maps.append({
            "edges": edges_c,
            "cba": np.ascontiguousarray(cba_c),
            "cbw": np.ascontiguousarray(cbw_c),
            "xT": xT_c,
            "ident2": ident2,
        })
    return in_maps, n_id.tolist(), caps.tolist(), tstart.tolist(), T, NOV, perm


def _build_program(n_id, caps, tstart, T, NOV):
    import concourse.tile as tile
    from concourse import bacc, mybir
    from contextlib import ExitStack

    f32 = mybir.dt.float32
    bf16 = mybir.dt.bfloat16
    fp8 = mybir.dt.float8e3
    nc = bacc.Bacc("TRN2", target_bir_lowering=False, debug=False,
                   num_devices=N_CORES)

    CWA = WIN + NOV + GROUP * WIN
    CWW = 3 * H + 1
    edges_ap = nc.dram_tensor("edges", [128, T, H], fp8, kind="ExternalInput").ap()
    cba_ap = nc.dram_tensor("cba", [128, CWA], bf16, kind="ExternalInput").ap()
    cbw_ap = nc.dram_tensor("cbw", [128, CWW], bf16, kind="ExternalInput").ap()
    xT_ap = nc.dram_tensor("xT", [H, NPAD], fp8, kind="ExternalInput").ap()
    ident2_ap = nc.dram_tensor("ident2", [128, WIN], fp8, kind="ExternalInput").ap()
    out_ap = nc.dram_tensor("out", [128, NBLK, H], bf16, kind="ExternalOutput").ap()

    n_groups = (NW + GROUP - 1) // GROUP

    with tile.TileContext(nc) as tc, ExitStack() as ctx:
        const = ctx.enter_context(tc.tile_pool(name="const", bufs=1))
        epool = ctx.enter_context(tc.tile_pool(name="edges", bufs=13))
        spool = ctx.enter_context(tc.tile_pool(name="sel", bufs=4))
        aggp = ctx.enter_context(tc.tile_pool(name="agg", bufs=1))
        hpool = ctx.enter_context(tc.tile_pool(name="h", bufs=2))
        opool = ctx.enter_context(tc.tile_pool(name="osb", bufs=2))
        pw = ctx.enter_context(tc.tile_pool(name="pw", bufs=4, space="PSUM"))
        ph = ctx.enter_context(tc.tile_pool(name="ph", bufs=1, space="PSUM"))
        po = ctx.enter_context(tc.tile_pool(name="po", bufs=3, space="PSUM"))

        # Queue plan.  A DMA_DIRECT2D enqueue OCCUPIES its engine until the
        # queue accepts it (~0.6-1.2us, longer when the queue is backed up),
        # so an engine that also runs per-group compute must carry only a
        # handful of early enqueues.  Sync (compute-free) carries the bulk
        # edge stream; Scalar carries 4 early const/x^T enqueues plus two
        # late edge groups enqueued INSIDE loop bodies (between computes);
        # gpsimd (software DGE) prefetches the 3 tail groups before its
        # output stores.
        n_groups_l = (NW + GROUP - 1) // GROUP
        etiles = {}

        def issue_edges(g, queue, first=False):
            wl = list(range(g * GROUP, min((g + 1) * GROUP, NW)))
            gt0 = tstart[wl[0]]
            gtiles = sum(caps[w] for w in wl)
            et = epool.tile([128, gtiles * H], fp8, tag="edges", name=f"et{g}")
            etiles[g] = et
            if first:
                c1 = caps[wl[0]]
                c3 = sum(caps[w] for w in wl[:4])
                queue.dma_start(
                    et[:, :c1 * H],
                    edges_ap[:, gt0:gt0 + c1, :].rearrange("p t h -> p (t h)"))
                queue.dma_start(
                    et[:, c1 * H:c3 * H],
                    edges_ap[:, gt0 + c1:gt0 + c3, :].rearrange("p t h -> p (t h)"))
                queue.dma_start(
                    et[:, c3 * H:],
                    edges_ap[:, gt0 + c3:gt0 + gtiles, :].rearrange("p t h -> p (t h)"))
            else:
                queue.dma_start(
                    et[:],
                    edges_ap[:, gt0:gt0 + gtiles, :].rearrange("p t h -> p (t h)"))

        N_SCALAR_EDGE = 2              # groups 9,10 enqueued in bodies 1,2
        N_GPSIMD_EDGE = 2              # groups 11,12 -- tail of the Sync queue

        issue_edges(0, nc.sync, first=True)
        cbw = const.tile([128, CWW], bf16)
        nc.sync.dma_start(cbw[:], cbw_ap[:])   # MLP weights beat MLP(0)
        for g in range(1, n_groups_l - N_SCALAR_EDGE - N_GPSIMD_EDGE):
            issue_edges(g, nc.sync)

        ident2_t = const.tile([128, WIN], fp8)
        nc.scalar.dma_start(ident2_t[:], ident2_ap[:])
        cba = const.tile([128, CWA], bf16)
        nc.scalar.dma_start(cba[:], cba_ap[:])
        xT_t = const.tile([H, NPAD], fp8)
        X1 = 2 * GROUP * WIN
        nc.scalar.dma_start(xT_t[:, :X1], xT_ap[:, :X1])
        nc.scalar.dma_start(xT_t[:, X1:], xT_ap[:, X1:])

        with tc.high_priority():
            for g in range(n_groups_l - N_GPSIMD_EDGE, n_groups_l):
                issue_edges(g, nc.gpsimd)

        B2O = WIN + NOV

        iota_t = cba[:, :WIN]
        colrel0 = WIN
        w1a_t = cbw[:, 0:H]
        w1b_t = cbw[:, H:2 * H]
        w2_t = cbw[:, 2 * H:3 * H]
        b1_t = cbw[:, 3 * H:3 * H + 1]

        zcol = const.tile([128, 1], bf16)
        nc.vector.memset(zcol[:], 0.0)
        aggT = aggp.tile([H, NPAD], bf16)

        def group_kov(g):
            return sum(caps[w] - n_id[w]
                       for w in range(g * GROUP, min((g + 1) * GROUP, NW)))

        # overflow-selector builds on VectorE hoisted 3 groups ahead: on
        # strict-FIFO engines nothing needed by a LATER group may queue
        # behind an instruction waiting on the CURRENT group's results.
        sels = {}
        ov_idx = 0

        def issue_sel(g):
            nonlocal ov_idx
            kov = group_kov(g)
            if not kov:
                sels[g] = None
                return
            Sall = spool.tile([128, kov * WIN], fp8, tag="S", name=f"sel{g}")
            nc.vector.scalar_tensor_tensor(
                out=Sall[:].rearrange("p (k w) -> p k w", k=kov),
                in0=iota_t.rearrange("p (k w) -> p k w", k=1)
                    .broadcast_to((128, kov, WIN)),
                scalar=0.0,
                in1=cba[:, colrel0 + ov_idx:colrel0 + ov_idx + kov]
                    .rearrange("p (k w) -> p k w", w=1)
                    .broadcast_to((128, kov, WIN)),
                op0=mybir.AluOpType.bypass,
                op1=mybir.AluOpType.is_equal)
            ov_idx += kov
            sels[g] = Sall

        for g in range(min(3, n_groups)):
            issue_sel(g)

        def emit_mlp(c0, ncols):
            """MLP for a 512-node group; runs one group behind the scatter
            so the aggT copy never sits on the TensorE critical path."""
            phh = ph.tile([H, ncols], f32, tag="ph")
            nc.tensor.matmul(phh[:], lhsT=w1a_t, rhs=xT_t[:, c0:c0 + ncols],
                             start=True, stop=False)
            nc.tensor.matmul(phh[:], lhsT=w1b_t, rhs=aggT[:, c0:c0 + ncols],
                             start=False, stop=True)
            # RELU in halves on VectorE into separate tiles: the W2 stage
            # only waits for the first half instead of the full activation
            half = min(2 * H, ncols)
            hTa = hpool.tile([H, half], bf16, tag="hTa", name="hTa")
            nc.vector.scalar_tensor_tensor(
                out=hTa[:], in0=phh[:, :half], scalar=b1_t,
                in1=zcol[:].broadcast_to((128, half)),
                op0=mybir.AluOpType.add, op1=mybir.AluOpType.max)
            if ncols > half:
                hTb = hpool.tile([H, ncols - half], bf16, tag="hTb", name="hTb")
                nc.vector.scalar_tensor_tensor(
                    out=hTb[:], in0=phh[:, half:], scalar=b1_t,
                    in1=zcol[:].broadcast_to((128, ncols - half)),
                    op0=mybir.AluOpType.add, op1=mybir.AluOpType.max)
            pog = po.tile([128, ncols], f32, tag="po")
            nblk = ncols // H
            for k in range(nblk):
                src = hTa[:, k * H:(k + 1) * H] if k * H < half else \
                    hTb[:, k * H - half:(k + 1) * H - half]
                nc.tensor.matmul(pog[:, k * H:(k + 1) * H],
                                 lhsT=src,
                                 rhs=w2_t, start=(k == 0), stop=(k == nblk - 1))
            # fused PSUM->SBUF copy + b2 bias add on DVE (gpsimd cannot
            # read PSUM); the output store follows on the gpsimd FIFO.
            osb = opool.tile([128, ncols], bf16, tag="osb")
            nc.vector.scalar_tensor_tensor(
                out=osb[:], in0=pog[:], scalar=0.0,
                in1=cba[:, B2O:B2O + ncols],
                op0=mybir.AluOpType.bypass, op1=mybir.AluOpType.add)
            nblk = ncols // H
            nc.gpsimd.dma_start(
                out_ap[:, c0 // H:c0 // H + nblk, :].rearrange("p t h -> p (t h)"),
                osb[:])

        pending_mlp = None
        scalar_edge = {1: n_groups - N_GPSIMD_EDGE - 2,
                       2: n_groups - N_GPSIMD_EDGE - 1}
        for g in range(n_groups):
            if g in scalar_edge:
                issue_edges(scalar_edge[g], nc.scalar)
            wlist = list(range(g * GROUP, min((g + 1) * GROUP, NW)))
            nwin = len(wlist)
            gt0 = tstart[wlist[0]]
            etile = etiles.pop(g)
            c0 = g * GROUP * WIN
            ncols = nwin * WIN
            if g + 3 < n_groups:
                issue_sel(g + 3)
            Sall = sels.pop(g)

            pwg = pw.tile([H, ncols], f32, tag="pw")
            sj = 0
            for wi, w in enumerate(wlist):
                t0 = tstart[w] - gt0           # tile offset within etile
                pslice = pwg[:, wi * WIN:(wi + 1) * WIN]
                for t in range(caps[w]):
                    lhsT = etile[:, (t0 + t) * H:(t0 + t + 1) * H]
                    if t < n_id[w]:
                        rhs = ident2_t[:]
                    else:
                        rhs = Sall[:, sj * WIN:(sj + 1) * WIN]
                        sj += 1
                    nc.tensor.matmul(out=pslice, lhsT=lhsT, rhs=rhs,
                                     start=(t == 0), stop=(t == caps[w] - 1))
            # MLP(g-1) BEFORE the aggT CAST(g): the dependency tracker works
            # at tile granularity on aggT, so a read emitted after CAST(g)
            # would wait for scatter(g) -- putting the CAST on the TensorE
            # critical path.
            if pending_mlp is not None:
                emit_mlp(*pending_mlp)
            pending_mlp = (c0, ncols)
            nc.scalar.copy(out=aggT[:, c0:c0 + ncols], in_=pwg[:])
        emit_mlp(*pending_mlp)

    nc.finalize()
    return nc


def kernel(x, edge_index, edge_attr, u=None, batch=None, W1=None, b1=None,
           W2=None, b2=None, **_unused):
    _install_axon_trace_shim()
    from concourse.bass_utils import run_bass_kernel_spmd

    in_maps, n_id, caps, tstart, T, NOV, perm = _prep_host(
        x, edge_index, edge_attr, W1, b1, W2, b2)
    nc = _build_program(n_id, caps, tstart, T, NOV)
    res = run_bass_kernel_spmd(nc, in_maps, core_ids=list(range(N_CORES)))
    parts = []
    for c in range(N_CORES):
        blk = np.asarray(res.results[c]["out"], np.float32)   # [128, NBLK, H]
        parts.append(blk.transpose(1, 0, 2).reshape(NPAD, H)[:NPC])
    out = np.empty((N_NODES, H), np.float32)
    out[perm] = np.concatenate(parts, axis=0)   # undo the degree sort
    return out


# revision 50
# speedup vs baseline: 1.0320x; 1.0320x over previous
"""GNN message-passing node model on 8 TRN2 NeuronCores.

Reference computation:
    agg = segment_sum(edge_attr, edge_index[1], num_segments=N)   # scatter-add
    h   = relu(concat([x, agg], 1) @ W1 + b1)
    out = h @ W2 + b2

Sharding: destination nodes are split into 8 contiguous blocks of 6250;
edges are partitioned by destination (per the sharding hint), so the
scatter-add is fully local per core -- no halo exchange.

Per core the segment-sum runs as dense TensorE matmuls over 128-edge tiles:
    aggT[128 feat, 64 nodes] += E_tile[128 edge, 128 feat]^T @ S_tile
with 64-node destination windows.  Each tile holds TWO rank levels of the
window (rank r edges at partitions 0-63, rank r+1 at 64-127), so the
selector for in-cap tiles is the constant stacked identity [I64; I64] and
each tile streams only 64 columns (the per-tile cost is LDWEIGHTS-bound,
~35 ns/tile with the fp8 fast-weight-load path).

Nodes are RELABELED on the host (x/out permuted to match): sorted by
descending degree into degree-homogeneous windows (~95% of edge-tile
slots used, nearly zero overflow tiles), then the windows are snake-dealt
across the 13 MLP groups so every group carries an equal share of edge
bytes (otherwise TensorE starves on the edge DMA early).  The rare
overflow edges get one-hot selectors built in bulk per group on VectorE
via is_equal(iota bcast, colrel bcast).

Edge values AND the resident x^T ride in fp8 e3m4 (x2 pre-scale, folded
back via W1b resp. W1a * 0.5 on the host -- exact exponent shifts): this
halves the dominant HBM traffic vs bf16 and keeps the end-to-end
relative error ~1.42e-2 (validated against the reference in numpy AND on
hardware; e3m4 has 4 mantissa bits, ample for N(0,1) data; W1a itself
must stay bf16 -- its ~+-0.06 entries fall into e3m4's subnormal range).
Selector products are exact (0/1 x fp8), accumulation is f32 PSUM, and
the W1a(bf16) x xT(fp8) matmul runs as a mixed-dtype matmul.

The MLP is software-pipelined ONE GROUP BEHIND the scatter, entirely
on-chip in bf16: the aggT PSUM->SBUF copy runs on ScalarE (off the
TensorE critical path), h = relu(W1a^T x^T + W1b'^T aggT + b1) with the
relu+bias as a single VectorE scalar_tensor_tensor (add, max 0) split in
halves so the W2 stage starts after half the activation, and the b2 bias
rides the PSUM->SBUF output copy (DVE add against a broadcast tile).

DMA plan (a hardware-DGE enqueue occupies its engine ~0.6-1.2 us, so the
compute-carrying Scalar engine gets only a handful): Sync carries the
bulk edge stream (groups 0-8) + MLP weights; Scalar carries consts +
resident x^T early plus two late edge groups enqueued between computes;
gpsimd (software DGE) prefetches the two tail groups and then streams the
outputs.  All 13 edge-group tiles are SBUF-resident (no ring-reuse waits).
The output stays partition-major [128, blk, H] in DRAM (fully contiguous
stores); the host transposes back to [node, feat] and undoes the degree
sort.
"""

import os
import sys
import types

import numpy as np
import ml_dtypes

N_NODES = 50000
N_EDGES = 600000
H = 128
N_CORES = 8
NPC = N_NODES // N_CORES          # 6250 nodes per core
WIN = 64                          # destination-node window (matmul N dim)
RANKS = 2                         # rank levels stacked per tile (64*2=128)
NW = (NPC + WIN - 1) // WIN       # 98 windows per core
NPAD = NW * WIN                   # 6272 padded nodes per core
NBLK = NPAD // H                  # 49 output blocks of 128 nodes
GROUP = 8                         # windows per MLP group (512 nodes)
KMAX = 48                         # max rank-cap considered
FP8_SCALE = 2.0                   # edge pre-scale (power of 2, exact fold)


def _install_axon_trace_shim():
    """If the harness sets BASS_TRACE=1, run_bass_kernel_spmd imports
    antenv.axon_hooks; slim axon containers lack it.  Provide the same
    ctypes-based NTFF hook trn_agent_boot would register, so tracing works
    instead of crashing.  No-op when the real module exists."""
    try:
        import antenv.axon_hooks  # noqa: F401
        return
    except ImportError:
        pass
    mod = types.ModuleType("antenv.axon_hooks")
    mod._hook = None
    mod.set_axon_ntff_profile_hook = lambda h: setattr(mod, "_hook", h)
    mod.get_axon_ntff_profile_hook = lambda: mod._hook
    sys.modules["antenv.axon_hooks"] = mod
    so_path = "/opt/axon/libaxon_pjrt.so"
    if os.path.exists(so_path):
        try:
            from trn_agent_boot.trn_boot import _ntff_profile_via_ctypes
            mod._hook = _ntff_profile_via_ctypes(so_path)
        except Exception:
            mod._hook = None
    try:
        from concourse import bass_utils
        _orig_upload = bass_utils.upload_artifacts

        def _safe_upload(tmpdir):
            try:
                return _orig_upload(tmpdir)
            except Exception as e:  # no bucket access in sandbox
                return f"upload-skipped({e.__class__.__name__})"

        bass_utils.upload_artifacts = _safe_upload
    except Exception:
        pass


def _prep_host(x, edge_index, edge_attr, W1, b1, W2, b2):
    """Rank-pack edges per (core, 64-node window); build per-core inputs."""
    fp8 = ml_dtypes.float8_e3m4
    bf16 = ml_dtypes.bfloat16

    col0 = np.asarray(edge_index)[1].astype(np.int64)
    deg0 = np.bincount(col0, minlength=N_NODES)
    # Permute nodes within each core (host-only relabeling; x/out are
    # permuted to match): sort by descending degree into degree-homogeneous
    # 64-node windows (~95% of edge slots used, almost no overflow tiles),
    # then snake-deal the windows across the MLP groups so every group
    # carries a near-equal share of edges -- otherwise the first groups
    # hold most of the bytes and TensorE starves on the edge DMA early on.
    n_groups_p = (NW + GROUP - 1) // GROUP
    gslots = [GROUP] * n_groups_p
    gslots[-1] = NW - GROUP * (n_groups_p - 1)
    perm = np.empty(N_NODES, np.int64)     # perm[new_id] = old_id
    for c in range(N_CORES):
        seg = slice(c * NPC, (c + 1) * NPC)
        order_nodes = np.argsort(-deg0[seg], kind="stable") + c * NPC
        wins = [order_nodes[64 * j:64 * j + 64] for j in range(NW)]
        partial = wins.pop()               # 42-node window pinned last
        ww = np.array([deg0[wn].sum() for wn in wins])
        worder = np.argsort(ww, kind="stable")          # light -> heavy
        groups = [[] for _ in range(n_groups_p)]
        gi, step = 0, 1
        for j in worder:                    # snake-deal for equal sums
            while len(groups[gi]) >= gslots[gi] - (1 if gi == n_groups_p - 1 else 0):
                gi += step
                if gi in (-1, n_groups_p):
                    step = -step
                    gi += step
            groups[gi].append(j)
            gi += step
            if gi in (-1, n_groups_p):
                step = -step
                gi += step
        dealt = [wins[j] for grp in groups for j in grp] + [partial]
        perm[seg] = np.concatenate(dealt)
    inv = np.empty(N_NODES, np.int64)
    inv[perm] = np.arange(N_NODES)
    col = inv[col0]
    core = col // NPC
    local = col - core * NPC               # 0..6249
    w = local // WIN                       # 0..97
    rel = local - w * WIN                  # 0..63

    # per-node degree and per-edge rank within its node
    deg = deg0[perm]
    node_start = np.concatenate([[0], np.cumsum(deg)[:-1]])
    order = np.argsort(col, kind="stable")         # edges grouped by node
    scol = col[order]
    rank = np.arange(N_EDGES, dtype=np.int64) - node_start[scol]

    # choose rank cap per window: minimize ceil(k/RANKS) + ceil(over/128)
    degs = np.zeros((N_CORES, NW * WIN), np.int64)
    degs[:, :NPC] = deg.reshape(N_CORES, NPC)
    degs = degs.reshape(N_CORES, NW, WIN)
    ks = np.arange(KMAX + 1)
    over = np.clip(degs[..., None] - ks, 0, None).sum(axis=2)   # [C, NW, K+1]
    over_max = over.max(axis=0)                                  # [NW, K+1]
    cap_ov_k = -(-over_max // 128)
    cost = -(-ks // RANKS) + cap_ov_k                            # [NW, K+1]
    kstar = np.zeros(NW, np.int64)
    for wi in range(NW):
        c = cost[wi]
        best = int(np.min(c))
        kstar[wi] = int(np.max(np.nonzero(c == best)[0]))
    cap_id = kstar                                    # rank cap (edges/node)
    n_id = -(-cap_id // RANKS)                        # identity tiles/window
    cap_ov = cap_ov_k[np.arange(NW), kstar]           # overflow tiles/window
    caps = np.maximum(1, n_id + cap_ov)               # total tiles/window
    tstart = np.concatenate([[0], np.cumsum(caps)[:-1]])
    T = int(caps.sum())

    # slot assignment (in node-sorted edge order)
    e_core = core[order]
    e_w = w[order]
    e_rel = rel[order]
    is_id = rank < cap_id[e_w]
    tile_g = np.empty(N_EDGES, np.int64)
    part = np.empty(N_EDGES, np.int64)
    tile_g[is_id] = tstart[e_w[is_id]] + rank[is_id] // RANKS
    part[is_id] = (rank[is_id] % RANKS) * WIN + e_rel[is_id]
    # overflow edges: ordinal within (core, window) block (node-order keeps
    # blocks contiguous)
    ovm = ~is_id
    ovkey = e_core[ovm] * NW + e_w[ovm]
    ovcnt = np.bincount(ovkey, minlength=N_CORES * NW)
    ovstart = np.concatenate([[0], np.cumsum(ovcnt)[:-1]])
    ov_rank = np.arange(int(ovm.sum()), dtype=np.int64) - ovstart[ovkey]
    tile_g[ovm] = tstart[e_w[ovm]] + n_id[e_w[ovm]] + ov_rank // 128
    part[ovm] = ov_rank % 128

    ea = np.clip(np.asarray(edge_attr, np.float32) * FP8_SCALE, -15.5, 15.5)
    ea = ea.astype(fp8)
    x = np.asarray(x, np.float32)
    W1 = np.asarray(W1, np.float32)
    b1 = np.asarray(b1, np.float32)
    W2 = np.asarray(W2, np.float32)
    b2 = np.asarray(b2, np.float32)

    iota = np.tile(np.arange(WIN, dtype=np.float32), (128, 1)).astype(bf16)
    ident2 = np.zeros((128, WIN), np.float32)
    ident2[np.arange(128), np.arange(128) % WIN] = 1.0   # [I64; I64]
    ident2 = ident2.astype(fp8)
    w1a = np.ascontiguousarray(W1[:H] * (1.0 / FP8_SCALE)).astype(bf16)
    w1b = np.ascontiguousarray(W1[H:] * (1.0 / FP8_SCALE)).astype(bf16)
    w2c = W2.astype(bf16)
    b1c = b1.reshape(H, 1).astype(bf16)                     # ACT bias column
    # b2 broadcast across partitions: fused into the PSUM->SBUF copy on DVE
    b2bc = np.tile(b2.reshape(1, H), (128, GROUP * WIN // H)).astype(bf16)

    # overflow tiles: global tile index -> compact overflow column index
    ov_tiles = []
    for wi in range(NW):
        for t in range(int(n_id[wi]), int(caps[wi])):
            ov_tiles.append(int(tstart[wi]) + t)
    NOV = max(1, len(ov_tiles))
    ov_col = np.full(T, -1, np.int64)
    for j, tg in enumerate(ov_tiles):
        ov_col[tg] = j

    in_maps = []
    for c in range(N_CORES):
        m = (e_core == c)
        edges_c = np.zeros((128, T, H), dtype=fp8)
        edges_c[part[m], tile_g[m], :] = ea[order[m]]
        colrel_c = np.full((128, NOV), 200.0, np.float32).astype(bf16)
        mo = m & ovm
        colrel_c[part[mo], ov_col[tile_g[mo]]] = e_rel[mo]
        # bf16 consts in wide blobs: hardware-DGE DMA cost is packet-count
        # bound (~128 packets per transfer regardless of width), so wide
        # transfers beat narrow ones ~8x.  cbA feeds the selector builds
        # (scalar queue, early); cbW feeds the MLP (sync queue, right after
        # group-0 edges so it beats MLP(0) despite queue contention).
        cba_c = np.concatenate([iota, colrel_c, b2bc], axis=1)
        cbw_c = np.concatenate([w1a, w1b, w2c, b1c], axis=1)
        xT_c = np.zeros((H, NPAD), dtype=fp8)
        xT_c[:, :NPC] = np.clip(
            x[perm[c * NPC:(c + 1) * NPC]].T * FP8_SCALE, -15.5, 15.5).astype(fp8)
        in_maps.append({
            "edges": edges_c,
            "cba": np.ascontiguousarray(cba_c),
            "cbw": np.ascontiguousarray(cbw_c),
            "xT": xT_c,
            "ident2": ident2,
        })
    return in_maps, n_id.tolist(), caps.tolist(), tstart.tolist(), T, NOV, perm


def _build_program(n_id, caps, tstart, T, NOV):
    import concourse.tile as tile
    from concourse import bacc, mybir
    from contextlib import ExitStack

    f32 = mybir.dt.float32
    bf16 = mybir.dt.bfloat16
    fp8 = mybir.dt.float8e3
    nc = bacc.Bacc("TRN2", target_bir_lowering=False, debug=False,
                   num_devices=N_CORES)

    CWA = WIN + NOV + GROUP * WIN
    CWW = 3 * H + 1
    edges_ap = nc.dram_tensor("edges", [128, T, H], fp8, kind="ExternalInput").ap()
    cba_ap = nc.dram_tensor("cba", [128, CWA], bf16, kind="ExternalInput").ap()
    cbw_ap = nc.dram_tensor("cbw", [128, CWW], bf16, kind="ExternalInput").ap()
    xT_ap = nc.dram_tensor("xT", [H, NPAD], fp8, kind="ExternalInput").ap()
    ident2_ap = nc.dram_tensor("ident2", [128, WIN], fp8, kind="ExternalInput").ap()
    out_ap = nc.dram_tensor("out", [128, NBLK, H], bf16, kind="ExternalOutput").ap()

    n_groups = (NW + GROUP - 1) // GROUP

    with tile.TileContext(nc) as tc, ExitStack() as ctx:
        const = ctx.enter_context(tc.tile_pool(name="const", bufs=1))
        epool = ctx.enter_context(tc.tile_pool(name="edges", bufs=13))
        spool = ctx.enter_context(tc.tile_pool(name="sel", bufs=4))
        aggp = ctx.enter_context(tc.tile_pool(name="agg", bufs=1))
        hpool = ctx.enter_context(tc.tile_pool(name="h", bufs=2))
        opool = ctx.enter_context(tc.tile_pool(name="osb", bufs=2))
        pw = ctx.enter_context(tc.tile_pool(name="pw", bufs=4, space="PSUM"))
        ph = ctx.enter_context(tc.tile_pool(name="ph", bufs=1, space="PSUM"))
        po = ctx.enter_context(tc.tile_pool(name="po", bufs=3, space="PSUM"))

        # Queue plan.  A DMA_DIRECT2D enqueue OCCUPIES its engine until the
        # queue accepts it (~0.6-1.2us, longer when the queue is backed up),
        # so an engine that also runs per-group compute must carry only a
        # handful of early enqueues.  Sync (compute-free) carries the bulk
        # edge stream; Scalar carries 4 early const/x^T enqueues plus two
        # late edge groups enqueued INSIDE loop bodies (between computes);
        # gpsimd (software DGE) prefetches the 3 tail groups before its
        # output stores.
        n_groups_l = (NW + GROUP - 1) // GROUP
        etiles = {}

        def issue_edges(g, queue, first=False):
            wl = list(range(g * GROUP, min((g + 1) * GROUP, NW)))
            gt0 = tstart[wl[0]]
            gtiles = sum(caps[w] for w in wl)
            et = epool.tile([128, gtiles * H], fp8, tag="edges", name=f"et{g}")
            etiles[g] = et
            if first:
                c1 = caps[wl[0]]
                c3 = sum(caps[w] for w in wl[:4])
                queue.dma_start(
                    et[:, :c1 * H],
                    edges_ap[:, gt0:gt0 + c1, :].rearrange("p t h -> p (t h)"))
                queue.dma_start(
                    et[:, c1 * H:c3 * H],
                    edges_ap[:, gt0 + c1:gt0 + c3, :].rearrange("p t h -> p (t h)"))
                queue.dma_start(
                    et[:, c3 * H:],
                    edges_ap[:, gt0 + c3:gt0 + gtiles, :].rearrange("p t h -> p (t h)"))
            else:
                queue.dma_start(
                    et[:],
                    edges_ap[:, gt0:gt0 + gtiles, :].rearrange("p t h -> p (t h)"))

        N_SCALAR_EDGE = 2              # groups 9,10 enqueued in bodies 1,2
        N_GPSIMD_EDGE = 2              # groups 11,12 -- tail of the Sync queue

        issue_edges(0, nc.sync, first=True)
        cbw = const.tile([128, CWW], bf16)
        nc.sync.dma_start(cbw[:], cbw_ap[:])   # MLP weights beat MLP(0)
        for g in range(1, n_groups_l - N_SCALAR_EDGE - N_GPSIMD_EDGE):
            issue_edges(g, nc.sync)

        ident2_t = const.tile([128, WIN], fp8)
        nc.scalar.dma_start(ident2_t[:], ident2_ap[:])
        cba = const.tile([128, CWA], bf16)
        nc.scalar.dma_start(cba[:], cba_ap[:])
        xT_t = const.tile([H, NPAD], fp8)
        X1 = 2 * GROUP * WIN
        nc.scalar.dma_start(xT_t[:, :X1], xT_ap[:, :X1])
        nc.scalar.dma_start(xT_t[:, X1:], xT_ap[:, X1:])

        with tc.high_priority():
            for g in range(n_groups_l - N_GPSIMD_EDGE, n_groups_l):
                issue_edges(g, nc.gpsimd)

        B2O = WIN + NOV

        iota_t = cba[:, :WIN]
        colrel0 = WIN
        w1a_t = cbw[:, 0:H]
        w1b_t = cbw[:, H:2 * H]
        w2_t = cbw[:, 2 * H:3 * H]
        b1_t = cbw[:, 3 * H:3 * H + 1]

        zcol = const.tile([128, 1], bf16)
        nc.vector.memset(zcol[:], 0.0)
        aggT = aggp.tile([H, NPAD], bf16)

        def group_kov(g):
            return sum(caps[w] - n_id[w]
                       for w in range(g * GROUP, min((g + 1) * GROUP, NW)))

        # overflow-selector builds on VectorE hoisted 3 groups ahead: on
        # strict-FIFO engines nothing needed by a LATER group may queue
        # behind an instruction waiting on the CURRENT group's results.
        sels = {}
        ov_idx = 0

        def issue_sel(g):
            nonlocal ov_idx
            kov = group_kov(g)
            if not kov:
                sels[g] = None
                return
            Sall = spool.tile([128, kov * WIN], fp8, tag="S", name=f"sel{g}")
            nc.vector.scalar_tensor_tensor(
                out=Sall[:].rearrange("p (k w) -> p k w", k=kov),
                in0=iota_t.rearrange("p (k w) -> p k w", k=1)
                    .broadcast_to((128, kov, WIN)),
                scalar=0.0,
                in1=cba[:, colrel0 + ov_idx:colrel0 + ov_idx + kov]
                    .rearrange("p (k w) -> p k w", w=1)
                    .broadcast_to((128, kov, WIN)),
                op0=mybir.AluOpType.bypass,
                op1=mybir.AluOpType.is_equal)
            ov_idx += kov
            sels[g] = Sall

        for g in range(min(3, n_groups)):
            issue_sel(g)

        def emit_mlp(c0, ncols):
            """MLP for a 512-node group; runs one group behind the scatter
            so the aggT copy never sits on the TensorE critical path."""
            phh = ph.tile([H, ncols], f32, tag="ph")
            nc.tensor.matmul(phh[:], lhsT=w1a_t, rhs=xT_t[:, c0:c0 + ncols],
                             start=True, stop=False)
            nc.tensor.matmul(phh[:], lhsT=w1b_t, rhs=aggT[:, c0:c0 + ncols],
                             start=False, stop=True)
            # RELU in halves on VectorE into separate tiles: the W2 stage
            # only waits for the first half instead of the full activation
            half = min(2 * H, ncols)
            hTa = hpool.tile([H, half], bf16, tag="hTa", name="hTa")
            nc.vector.scalar_tensor_tensor(
                out=hTa[:], in0=phh[:, :half], scalar=b1_t,
                in1=zcol[:].broadcast_to((128, half)),
                op0=mybir.AluOpType.add, op1=mybir.AluOpType.max)
            if ncols > half:
                hTb = hpool.tile([H, ncols - half], bf16, tag="hTb", name="hTb")
                nc.vector.scalar_tensor_tensor(
                    out=hTb[:], in0=phh[:, half:], scalar=b1_t,
                    in1=zcol[:].broadcast_to((128, ncols - half)),
                    op0=mybir.AluOpType.add, op1=mybir.AluOpType.max)
            pog = po.tile([128, ncols], f32, tag="po")
            nblk = ncols // H
            for k in range(nblk):
                src = hTa[:, k * H:(k + 1) * H] if k * H < half else \
                    hTb[:, k * H - half:(k + 1) * H - half]
                nc.tensor.matmul(pog[:, k * H:(k + 1) * H],
                                 lhsT=src,
                                 rhs=w2_t, start=(k == 0), stop=(k == nblk - 1))
            # fused PSUM->SBUF copy + b2 bias add on DVE (gpsimd cannot
            # read PSUM); the output store follows on the gpsimd FIFO.
            osb = opool.tile([128, ncols], bf16, tag="osb")
            nc.vector.scalar_tensor_tensor(
                out=osb[:], in0=pog[:], scalar=0.0,
                in1=cba[:, B2O:B2O + ncols],
                op0=mybir.AluOpType.bypass, op1=mybir.AluOpType.add)
            nblk = ncols // H
            outq = nc.scalar if c0 // H + nblk == NBLK else nc.gpsimd
            outq.dma_start(
                out_ap[:, c0 // H:c0 // H + nblk, :].rearrange("p t h -> p (t h)"),
                osb[:])

        pending_mlp = None
        scalar_edge = {1: n_groups - N_GPSIMD_EDGE - 2,
                       2: n_groups - N_GPSIMD_EDGE - 1}
        for g in range(n_groups):
            if g in scalar_edge:
                issue_edges(scalar_edge[g], nc.scalar)
            wlist = list(range(g * GROUP, min((g + 1) * GROUP, NW)))
            nwin = len(wlist)
            gt0 = tstart[wlist[0]]
            etile = etiles.pop(g)
            c0 = g * GROUP * WIN
            ncols = nwin * WIN
            if g + 3 < n_groups:
                issue_sel(g + 3)
            Sall = sels.pop(g)

            pwg = pw.tile([H, ncols], f32, tag="pw")
            sj = 0
            for wi, w in enumerate(wlist):
                t0 = tstart[w] - gt0           # tile offset within etile
                pslice = pwg[:, wi * WIN:(wi + 1) * WIN]
                for t in range(caps[w]):
                    lhsT = etile[:, (t0 + t) * H:(t0 + t + 1) * H]
                    if t < n_id[w]:
                        rhs = ident2_t[:]
                    else:
                        rhs = Sall[:, sj * WIN:(sj + 1) * WIN]
                        sj += 1
                    nc.tensor.matmul(out=pslice, lhsT=lhsT, rhs=rhs,
                                     start=(t == 0), stop=(t == caps[w] - 1))
            # MLP(g-1) BEFORE the aggT CAST(g): the dependency tracker works
            # at tile granularity on aggT, so a read emitted after CAST(g)
            # would wait for scatter(g) -- putting the CAST on the TensorE
            # critical path.
            if pending_mlp is not None:
                emit_mlp(*pending_mlp)
            pending_mlp = (c0, ncols)
            nc.scalar.copy(out=aggT[:, c0:c0 + ncols], in_=pwg[:])
        emit_mlp(*pending_mlp)

    nc.finalize()
    return nc


def kernel(x, edge_index, edge_attr, u=None, batch=None, W1=None, b1=None,
           W2=None, b2=None, **_unused):
    _install_axon_trace_shim()
    from concourse.bass_utils import run_bass_kernel_spmd

    in_maps, n_id, caps, tstart, T, NOV, perm = _prep_host(
        x, edge_index, edge_attr, W1, b1, W2, b2)
    nc = _build_program(n_id, caps, tstart, T, NOV)
    res = run_bass_kernel_spmd(nc, in_maps, core_ids=list(range(N_CORES)))
    parts = []
    for c in range(N_CORES):
        blk = np.asarray(res.results[c]["out"], np.float32)   # [128, NBLK, H]
        parts.append(blk.transpose(1, 0, 2).reshape(NPAD, H)[:NPC])
    out = np.empty((N_NODES, H), np.float32)
    out[perm] = np.concatenate(parts, axis=0)   # undo the degree sort
    return out
